# revision 3
# baseline (speedup 1.0000x reference)
"""Trainium2 Bass kernel for nn_A3TGCN2_EdgeClassifier (GNN message passing).

Math (validated vs reference in fp32, rel err ~4e-7): with H0 = 0 the GRU
collapses (R drops out; softmax over one period == 1):
    deg[d] = 1 + sum_{e: dst=d} ew[e];   dinv = deg^-1/2
    Y  = X @ [Wz@lzW[:64] | Wh@lhW[:64]]              (N,128)
    Ys = dinv * Y
    Yagg[d] = dinv[d] * ( sum_e ew[e]*Ys[src[e]] + Ys[d] )
    Z = sigmoid(Yagg[:,:64] + bz');  Ht = tanh(Yagg[:,64:] + bh')
    h = (1-Z)*Ht;  A = h@W1[:64];  B = h@W1[64:] + b1
    out[e] = relu(A[esrc]+B[edst]) @ W2 + b2          (E,2)

Distribution: nodes sharded across 8 cores (12544 each); each core builds
its slice of the (N,128) node tables, all-gathered via collectives; the
GCN aggregation is sharded by dst ownership and uses indirect-DMA row
gathers (128 rows/call, offsets staged at 32B-aligned columns — the only
descriptor pattern this runtime executes correctly) plus a one-hot matmul
scatter (segment-sum accumulated in PSUM, no RMW). The edge MLP is
edge-sharded contiguously; A/B rows are fetched as 128B half-rows.
Chunk capacities are maxed across cores so the single SPMD graph is
identical on every core.
"""

import sys

try:
    import concourse.bass as bass  # noqa: F401
except Exception:  # pragma: no cover
    sys.path.insert(0, "/opt/trn_rl_repo")

import numpy as np
import ml_dtypes

import concourse.bass as bass
import concourse.mybir as mybir
from concourse import bacc, tile
from concourse.bass_utils import run_bass_kernel_spmd

BF16 = ml_dtypes.bfloat16
F32 = np.float32

NCORES = 8
N = 100_000
E = 1_600_000
FIN = 80
NLOC = 12544               # padded nodes per core
NPAD = NLOC * NCORES       # 100352
TPC = NLOC // 128          # 98 node tiles per core
GT = 6                     # node tiles per aggregation group (PSUM banks)
PSZ = 8192                 # edges per MLP piece
EPC = E // NCORES          # 200000
ECHUNKS = (EPC + 127) // 128   # 1563
STOT = ECHUNKS * 128           # 200064
SP = 8                     # offset column spacing (32B) for indirect DMA

dt = mybir.dt


def _prep(inputs):
    """Host-side sharding / graph preprocessing. Returns (in_maps, meta)."""
    x = np.asarray(inputs["x"], F32)[0, :, :, 0]
    ei = np.asarray(inputs["edge_index"]).astype(np.int64)
    src, dst = ei[0], ei[1]
    ew = np.asarray(inputs["edge_weight"], F32)
    esrc = np.asarray(inputs["edge_src"]).astype(np.int64)
    edst = np.asarray(inputs["edge_dst"]).astype(np.int64)

    lzW = np.asarray(inputs["lz_W"], F32)[:64]
    lhW = np.asarray(inputs["lh_W"], F32)[:64]
    Wzp = np.asarray(inputs["Wz"], F32) @ lzW
    Whp = np.asarray(inputs["Wh"], F32) @ lhW
    bzp = np.asarray(inputs["bz"], F32) @ lzW + np.asarray(inputs["lz_b"], F32)
    bhp = np.asarray(inputs["bh"], F32) @ lhW + np.asarray(inputs["lh_b"], F32)
    W1 = np.asarray(inputs["mlp_W1"], F32)
    b1 = np.asarray(inputs["mlp_b1"], F32)
    W2 = np.asarray(inputs["mlp_W2"], F32)
    b2 = np.asarray(inputs["mlp_b2"], F32)

    wfused = np.ascontiguousarray(np.concatenate([Wzp, Whp], 1)).astype(BF16)
    gbias = np.tile(np.concatenate([bzp, bhp])[None, :], (128, 1)).astype(F32)
    w1cat = np.ascontiguousarray(np.concatenate([W1[:64], W1[64:]], 1)).astype(BF16)
    b1row = np.concatenate([np.zeros(64, F32), b1])[None, :].astype(BF16)
    w2sb = W2.astype(BF16)
    b2rep = np.tile(b2[None, :], (128, 64)).astype(F32)
    iota = np.tile(np.arange(128, dtype=F32)[None, :], (128, 1))
    ident = np.eye(128, dtype=F32).astype(BF16)
    ones1 = np.ones((1, 128), BF16)

    xpad = np.zeros((NPAD, FIN), F32)
    xpad[:N] = x
    xts = [np.ascontiguousarray(xpad[k * NLOC:(k + 1) * NLOC].T).astype(BF16)
           for k in range(NCORES)]

    counts = np.bincount(dst, minlength=NPAD)
    L = int(counts.max())
    order = np.argsort(dst, kind="stable")
    dsts = dst[order]
    ews = ew[order]
    srcs = src[order]
    cum = np.zeros(NPAD + 1, np.int64)
    cum[1:] = np.cumsum(counts)
    posn = np.arange(E) - cum[dsts]
    ewpad = np.zeros((NPAD, L), F32)
    ewpad[dsts, posn] = ews
    ewpads = [np.ascontiguousarray(
        ewpad[k * NLOC:(k + 1) * NLOC]
        .reshape(TPC, 128, L).transpose(1, 0, 2).reshape(128, TPC * L))
        for k in range(NCORES)]

    # ---- aggregation streams (sorted by dst, sharded by dst owner) ----
    k_of = dsts // NLOC
    t_of = (dsts % NLOC) // 128
    cnt = np.zeros((NCORES, TPC), np.int64)
    np.add.at(cnt, (k_of, t_of), 1)
    caps = np.maximum((cnt.max(0) + 127) // 128, 1)

    ngroups = (TPC + GT - 1) // GT
    groups = [list(range(g * GT, min((g + 1) * GT, TPC))) for g in range(ngroups)]
    colof = np.zeros(TPC, np.int64)
    gcol = []
    c = 0
    for grp in groups:
        c0 = c
        for t in grp:
            colof[t] = c
            c += int(caps[t])
        gcol.append((c0, c - c0))
    CTOT = c

    agg = []
    for k in range(NCORES):
        sel = slice(cum[k * NLOC], cum[(k + 1) * NLOC])
        d_k = dsts[sel]
        s_k = srcs[sel]
        e_k = ews[sel]
        t_k = (d_k - k * NLOC) // 128
        bstart = np.zeros(TPC, np.int64)
        bstart[1:] = np.cumsum(cnt[k])[:-1]
        pos = np.arange(len(t_k)) - bstart[t_k]
        col = colof[t_k] + pos // 128
        par = pos % 128
        gidx = np.zeros((128, SP * CTOT), np.int32)
        dstrel = np.zeros((128, CTOT), F32)
        ewagg = np.zeros((128, CTOT), F32)
        gidx[par, SP * col] = s_k.astype(np.int32)
        dstrel[par, col] = ((d_k - k * NLOC) % 128).astype(F32)
        ewagg[par, col] = e_k
        agg.append((gidx, dstrel, ewagg))

    # ---- MLP streams: contiguous shard; offsets into (2*NPAD, 64) view ----
    mlp = []
    for k in range(NCORES):
        s = np.zeros(STOT, np.int64)
        d = np.zeros(STOT, np.int64)
        s[:EPC] = esrc[k * EPC:(k + 1) * EPC]
        d[:EPC] = edst[k * EPC:(k + 1) * EPC]
        ms = np.zeros((128, SP * ECHUNKS), np.int32)
        md = np.zeros((128, SP * ECHUNKS), np.int32)
        # stream position i = c*128 + p
        ms[:, ::SP] = (2 * s).reshape(ECHUNKS, 128).T
        md[:, ::SP] = (2 * d + 1).reshape(ECHUNKS, 128).T
        mlp.append((ms, md))

    pieces = []
    c = 0
    while c < ECHUNKS:
        kk = min(PSZ // 128, ECHUNKS - c)
        pieces.append((c, kk))
        c += kk
    NP = len(pieces)

    consts = dict(wfused=wfused, gbias=gbias, w1cat=w1cat, b1row=b1row,
                  w2sb=w2sb, b2rep=b2rep, iota=iota, ident=ident, ones1=ones1)
    in_maps = []
    for k in range(NCORES):
        gidx, dstrel, ewagg = agg[k]
        midxs, midxd = mlp[k]
        in_maps.append(dict(
            xt=xts[k], ewpad=ewpads[k], gidx=gidx, dstrel=dstrel, ewagg=ewagg,
            midxs=midxs, midxd=midxd, **consts))

    meta = dict(L=L, caps=caps, groups=groups, colof=colof, gcol=gcol,
                CTOT=CTOT, pieces=pieces, NP=NP)
    return in_maps, meta


def _build(meta):
    L = meta["L"]
    caps = meta["caps"]
    groups = meta["groups"]
    colof = meta["colof"]
    gcol = meta["gcol"]
    CTOT = meta["CTOT"]
    pieces = meta["pieces"]
    NP = meta["NP"]

    nc = bacc.Bacc("TRN2", target_bir_lowering=False, debug=False,
                   num_devices=NCORES)

    xt_d = nc.dram_tensor("xt", [FIN, NLOC], dt.bfloat16, kind="ExternalInput")
    ewpad_d = nc.dram_tensor("ewpad", [128, TPC * L], dt.float32, kind="ExternalInput")
    gidx_d = nc.dram_tensor("gidx", [128, SP * CTOT], dt.int32, kind="ExternalInput")
    dstrel_d = nc.dram_tensor("dstrel", [128, CTOT], dt.float32, kind="ExternalInput")
    ewagg_d = nc.dram_tensor("ewagg", [128, CTOT], dt.float32, kind="ExternalInput")
    midxs_d = nc.dram_tensor("midxs", [128, SP * ECHUNKS], dt.int32, kind="ExternalInput")
    midxd_d = nc.dram_tensor("midxd", [128, SP * ECHUNKS], dt.int32, kind="ExternalInput")
    wfused_d = nc.dram_tensor("wfused", [FIN, 128], dt.bfloat16, kind="ExternalInput")
    gbias_d = nc.dram_tensor("gbias", [128, 128], dt.float32, kind="ExternalInput")
    w1cat_d = nc.dram_tensor("w1cat", [64, 128], dt.bfloat16, kind="ExternalInput")
    b1row_d = nc.dram_tensor("b1row", [1, 128], dt.bfloat16, kind="ExternalInput")
    w2sb_d = nc.dram_tensor("w2sb", [64, 2], dt.bfloat16, kind="ExternalInput")
    b2rep_d = nc.dram_tensor("b2rep", [128, 128], dt.float32, kind="ExternalInput")
    iota_d = nc.dram_tensor("iota", [128, 128], dt.float32, kind="ExternalInput")
    ident_d = nc.dram_tensor("ident", [128, 128], dt.bfloat16, kind="ExternalInput")
    ones1_d = nc.dram_tensor("ones1", [1, 128], dt.bfloat16, kind="ExternalInput")
    out_d = nc.dram_tensor("out", [NP * 128, 128], dt.float32, kind="ExternalOutput")

    ysloc = nc.dram_tensor("ysloc", [NLOC, 128], dt.bfloat16)
    ysfull = nc.dram_tensor("ysfull", [NPAD, 128], dt.bfloat16, addr_space="Shared")
    abloc = nc.dram_tensor("abloc", [NLOC, 128], dt.bfloat16)
    abfull = nc.dram_tensor("abfull", [NPAD, 128], dt.bfloat16, addr_space="Shared")

    rg = [list(range(NCORES))]
    IOA = bass.IndirectOffsetOnAxis

    with tile.TileContext(nc) as tc:
        with (
            tc.tile_pool(name="const", bufs=1) as cp,
            tc.tile_pool(name="persist", bufs=1) as pp,
        ):
            wfused_sb = cp.tile([FIN, 128], dt.bfloat16)
            gbias_sb = cp.tile([128, 128], dt.float32)
            w1cat_sb = cp.tile([64, 128], dt.bfloat16)
            b1row_sb = cp.tile([1, 128], dt.bfloat16)
            w2_sb = cp.tile([64, 2], dt.bfloat16)
            b2rep_sb = cp.tile([128, 128], dt.float32)
            iota_sb = cp.tile([128, 128], dt.float32)
            ident_sb = cp.tile([128, 128], dt.bfloat16)
            ones1_sb = cp.tile([1, 128], dt.bfloat16)
            for sb, d in [
                (wfused_sb, wfused_d), (gbias_sb, gbias_d), (w1cat_sb, w1cat_d),
                (b1row_sb, b1row_d), (w2_sb, w2sb_d), (b2rep_sb, b2rep_d),
                (iota_sb, iota_d), (ident_sb, ident_d), (ones1_sb, ones1_d),
            ]:
                nc.sync.dma_start(out=sb[:], in_=d[:])

            dinv = pp.tile([128, TPC], dt.float32)
            dinv2 = pp.tile([128, TPC], dt.float32)
            ys2b = pp.tile([128, TPC * 128], dt.bfloat16)

            # ---------------- deg / dinv ----------------
            with tc.tile_pool(name="degp", bufs=1) as dp:
                ewpad_sb = dp.tile([128, TPC * L], dt.float32)
                nc.sync.dma_start(out=ewpad_sb[:], in_=ewpad_d[:])
                deg = dp.tile([128, TPC], dt.float32)
                nc.vector.tensor_reduce(
                    deg[:], ewpad_sb[:].rearrange("p (t l) -> p t l", t=TPC),
                    axis=mybir.AxisListType.X, op=mybir.AluOpType.add)
                sq = dp.tile([128, TPC], dt.float32)
                nc.scalar.activation(sq[:], deg[:],
                                     mybir.ActivationFunctionType.Sqrt, bias=1.0)
                nc.vector.reciprocal(dinv[:], sq[:])
                nc.vector.tensor_mul(dinv2[:], dinv[:], dinv[:])

            # ---------------- node phase ----------------
            with (
                tc.tile_pool(name="xtp", bufs=3) as xtp,
                tc.tile_pool(name="ysp", bufs=3) as ysp,
                tc.tile_pool(name="pY", bufs=2, space="PSUM") as pYp,
            ):
                for t in range(TPC):
                    xt_t = xtp.tile([FIN, 128], dt.bfloat16)
                    nc.sync.dma_start(out=xt_t[:], in_=xt_d[:, t * 128:(t + 1) * 128])
                    pY = pYp.tile([128, 128], dt.float32)
                    nc.tensor.matmul(pY[:], xt_t[:], wfused_sb[:], start=True, stop=True)
                    ys_t = ysp.tile([128, 128], dt.bfloat16)
                    nc.vector.tensor_scalar_mul(ys_t[:], pY[:], dinv[:, t:t + 1])
                    nc.vector.scalar_tensor_tensor(
                        ys2b[:, t * 128:(t + 1) * 128], pY[:], dinv2[:, t:t + 1],
                        gbias_sb[:], op0=mybir.AluOpType.mult, op1=mybir.AluOpType.add)
                    nc.sync.dma_start(out=ysloc[t * 128:(t + 1) * 128, :], in_=ys_t[:])

            nc.gpsimd.collective_compute(
                "AllGather", mybir.AluOpType.bypass, replica_groups=rg,
                ins=[ysloc[:]], outs=[ysfull[:]])

            # ---------------- aggregation + gates + A|B ----------------
            with (
                tc.tile_pool(name="aggstream", bufs=1) as asp,
                tc.tile_pool(name="goff", bufs=3) as gop,
                tc.tile_pool(name="gat", bufs=10) as gatp,
                tc.tile_pool(name="oh", bufs=8) as ohp,
                tc.tile_pool(name="gate", bufs=3) as gp_,
                tc.tile_pool(name="abp", bufs=3) as abp,
                tc.tile_pool(name="pAgg", bufs=GT, space="PSUM") as pAgg,
                tc.tile_pool(name="pT", bufs=1, space="PSUM") as pT,
                tc.tile_pool(name="pAB", bufs=1, space="PSUM") as pAB,
            ):
                dstrel_sb = asp.tile([128, CTOT], dt.float32)
                ewagg_sb = asp.tile([128, CTOT], dt.float32)
                nc.sync.dma_start(out=dstrel_sb[:], in_=dstrel_d[:])
                nc.sync.dma_start(out=ewagg_sb[:], in_=ewagg_d[:])

                for g, grp in enumerate(groups):
                    c0, Kg = gcol[g]
                    goff = gop.tile([128, SP * Kg], dt.int32, tag="goff")
                    nc.sync.dma_start(
                        out=goff[:], in_=gidx_d[:, SP * c0:SP * (c0 + Kg)])
                    for t in grp:
                        ps = pAgg.tile([128, 128], dt.float32, tag="aggpsum")
                        nch = int(caps[t])
                        for cc in range(nch):
                            col = int(colof[t]) + cc
                            lc = col - c0
                            gt_ = gatp.tile([128, 128], dt.bfloat16, tag="gat")
                            nc.gpsimd.indirect_dma_start(
                                out=gt_[:], out_offset=None, in_=ysfull[:],
                                in_offset=IOA(ap=goff[:, SP * lc:SP * lc + 1],
                                              axis=0))
                            oh = ohp.tile([128, 128], dt.bfloat16)
                            nc.vector.tensor_scalar(
                                oh[:], iota_sb[:], dstrel_sb[:, col:col + 1],
                                ewagg_sb[:, col:col + 1],
                                op0=mybir.AluOpType.is_equal,
                                op1=mybir.AluOpType.mult)
                            nc.tensor.matmul(ps[:], oh[:], gt_[:],
                                             start=cc == 0, stop=cc == nch - 1)
                        yagg = gp_.tile([128, 128], dt.float32, tag="yagg")
                        nc.vector.scalar_tensor_tensor(
                            yagg[:], ps[:], dinv[:, t:t + 1],
                            ys2b[:, t * 128:(t + 1) * 128],
                            op0=mybir.AluOpType.mult, op1=mybir.AluOpType.add)
                        zt = gp_.tile([128, 64], dt.float32, tag="zt")
                        ht = gp_.tile([128, 64], dt.float32, tag="ht")
                        nc.scalar.activation(zt[:], yagg[:, 0:64],
                                             mybir.ActivationFunctionType.Sigmoid)
                        nc.scalar.activation(ht[:], yagg[:, 64:128],
                                             mybir.ActivationFunctionType.Tanh)
                        zh = gp_.tile([128, 64], dt.float32, tag="zh")
                        nc.vector.tensor_mul(zh[:], zt[:], ht[:])
                        hbf = gp_.tile([128, 64], dt.bfloat16, tag="hbf")
                        nc.vector.tensor_sub(hbf[:], ht[:], zh[:])
                        psT = pT.tile([64, 128], dt.bfloat16)
                        nc.tensor.transpose(psT[:], hbf[:], ident_sb[:])
                        hT = gp_.tile([64, 128], dt.bfloat16, tag="hT")
                        nc.vector.tensor_copy(hT[:], psT[:])
                        psAB = pAB.tile([128, 128], dt.float32)
                        nc.tensor.matmul(psAB[:], ones1_sb[:], b1row_sb[:],
                                         start=True, stop=False)
                        nc.tensor.matmul(psAB[:], hT[:], w1cat_sb[:],
                                         start=False, stop=True)
                        ab = abp.tile([128, 128], dt.bfloat16)
                        nc.scalar.copy(ab[:], psAB[:])
                        nc.sync.dma_start(out=abloc[t * 128:(t + 1) * 128, :],
                                          in_=ab[:])

            nc.gpsimd.collective_compute(
                "AllGather", mybir.AluOpType.bypass, replica_groups=rg,
                ins=[abloc[:]], outs=[abfull[:]])

            # ---------------- MLP phase ----------------
            abhalf = abfull[:].rearrange("n (a b) -> (n a) b", a=2)
            with (
                tc.tile_pool(name="moff", bufs=3) as mop,
                tc.tile_pool(name="sgp", bufs=10) as sgp,
                tc.tile_pool(name="hp", bufs=8) as hp_,
                tc.tile_pool(name="hT2", bufs=8) as hT2p,
                tc.tile_pool(name="op", bufs=3) as op_,
                tc.tile_pool(name="pO", bufs=2, space="PSUM") as pOp,
                tc.tile_pool(name="pT2", bufs=4, space="PSUM") as pT2p,
            ):
                for q, (ch0, kk) in enumerate(pieces):
                    moffs = mop.tile([128, SP * (PSZ // 128)], dt.int32, tag="ms")
                    moffd = mop.tile([128, SP * (PSZ // 128)], dt.int32, tag="md")
                    nc.sync.dma_start(out=moffs[:, :SP * kk],
                                      in_=midxs_d[:, SP * ch0:SP * (ch0 + kk)])
                    nc.sync.dma_start(out=moffd[:, :SP * kk],
                                      in_=midxd_d[:, SP * ch0:SP * (ch0 + kk)])
                    pO = pOp.tile([128, 128], dt.float32)
                    for b in range(kk):
                        ag = sgp.tile([128, 64], dt.bfloat16, tag="ag")
                        bg = sgp.tile([128, 64], dt.bfloat16, tag="bg")
                        nc.gpsimd.indirect_dma_start(
                            out=ag[:], out_offset=None, in_=abhalf,
                            in_offset=IOA(ap=moffs[:, SP * b:SP * b + 1], axis=0))
                        nc.gpsimd.indirect_dma_start(
                            out=bg[:], out_offset=None, in_=abhalf,
                            in_offset=IOA(ap=moffd[:, SP * b:SP * b + 1], axis=0))
                        hpre = hp_.tile([128, 64], dt.bfloat16, tag="hpre")
                        nc.vector.tensor_add(hpre[:], ag[:], bg[:])
                        psT2 = pT2p.tile([64, 128], dt.bfloat16)
                        nc.tensor.transpose(psT2[:], hpre[:], ident_sb[:])
                        hTm = hT2p.tile([64, 128], dt.bfloat16)
                        nc.scalar.activation(hTm[:], psT2[:],
                                             mybir.ActivationFunctionType.Relu)
                        nc.tensor.matmul(pO[:, 2 * b:2 * b + 2], hTm[:], w2_sb[:],
                                         start=True, stop=True)
                    osb = op_.tile([128, 128], dt.float32)
                    nc.vector.tensor_add(osb[:, :2 * kk], pO[:, :2 * kk],
                                         b2rep_sb[:, :2 * kk])
                    nc.sync.dma_start(out=out_d[q * 128:(q + 1) * 128, :],
                                      in_=osb[:])

    nc.compile()
    return nc


def _unshard(results, meta):
    pieces = meta["pieces"]
    out = np.zeros((E, 2), F32)
    for k in range(NCORES):
        outd = np.asarray(results[k]["out"])
        stream = np.empty((STOT, 2), F32)
        for q, (ch0, kk) in enumerate(pieces):
            blk = outd[q * 128:(q + 1) * 128, :2 * kk]
            stream[ch0 * 128:(ch0 + kk) * 128] = (
                blk.reshape(128, kk, 2).transpose(1, 0, 2).reshape(kk * 128, 2))
        out[k * EPC:(k + 1) * EPC] = stream[:EPC]
    return out


def kernel(**inputs):
    in_maps, meta = _prep(inputs)
    nc = _build(meta)
    res = run_bass_kernel_spmd(nc, in_maps, list(range(NCORES)))
    return _unshard(res.results, meta)


# revision 8
# speedup vs baseline: 1.1962x; 1.1962x over previous
"""Trainium2 Bass kernel for nn_A3TGCN2_EdgeClassifier (GNN message passing).

Math (validated vs reference in fp32, rel err ~4e-7): with H0 = 0 the GRU
collapses (R drops out; softmax over one period == 1):
    deg[d] = 1 + sum_{e: dst=d} ew[e];   dinv = deg^-1/2
    Y  = X @ [Wz@lzW[:64] | Wh@lhW[:64]]              (N,128)
    Ys = dinv * Y
    Yagg[d] = dinv[d] * ( sum_e ew[e]*Ys[src[e]] + Ys[d] )
    Z = sigmoid(Yagg[:,:64] + bz');  Ht = tanh(Yagg[:,64:] + bh')
    h = (1-Z)*Ht;  A = h@W1[:64];  B = h@W1[64:] + b1
    out[e] = relu(A[esrc]+B[edst]) @ W2 + b2          (E,2)

Distribution: nodes sharded across 8 cores (12544 each); each core builds
its slice of the (N,128) node tables, all-gathered via collectives; the
GCN aggregation is sharded by dst ownership and uses indirect-DMA row
gathers (128 rows/call, offsets staged at 32B-aligned columns — the only
descriptor pattern this runtime executes correctly) plus a one-hot matmul
scatter (segment-sum accumulated in PSUM, no RMW). The edge MLP is
edge-sharded contiguously; A/B rows are fetched as 128B half-rows.
Chunk capacities are maxed across cores so the single SPMD graph is
identical on every core.
"""

import sys

try:
    import concourse.bass as bass  # noqa: F401
except Exception:  # pragma: no cover
    sys.path.insert(0, "/opt/trn_rl_repo")

import numpy as np
import ml_dtypes

import concourse.bass as bass
import concourse.mybir as mybir
from concourse import bacc, tile
from concourse.bass_utils import run_bass_kernel_spmd

BF16 = ml_dtypes.bfloat16
F32 = np.float32

NCORES = 8
N = 100_000
E = 1_600_000
FIN = 80
NLOC = 12544               # padded nodes per core
NPAD = NLOC * NCORES       # 100352
TPC = NLOC // 128          # 98 node tiles per core
GT = 6                     # node tiles per aggregation group (PSUM banks)
PSZ = 8192                 # edges per MLP piece
EPC = E // NCORES          # 200000
ECHUNKS = (EPC + 127) // 128   # 1563
STOT = ECHUNKS * 128           # 200064
SP = 8                     # offset column spacing (32B) for indirect DMA

dt = mybir.dt


def _prep(inputs):
    """Host-side sharding / graph preprocessing. Returns (in_maps, meta)."""
    x = np.asarray(inputs["x"], F32)[0, :, :, 0]
    ei = np.asarray(inputs["edge_index"]).astype(np.int64)
    src, dst = ei[0], ei[1]
    ew = np.asarray(inputs["edge_weight"], F32)
    esrc = np.asarray(inputs["edge_src"]).astype(np.int64)
    edst = np.asarray(inputs["edge_dst"]).astype(np.int64)

    lzW = np.asarray(inputs["lz_W"], F32)[:64]
    lhW = np.asarray(inputs["lh_W"], F32)[:64]
    Wzp = np.asarray(inputs["Wz"], F32) @ lzW
    Whp = np.asarray(inputs["Wh"], F32) @ lhW
    bzp = np.asarray(inputs["bz"], F32) @ lzW + np.asarray(inputs["lz_b"], F32)
    bhp = np.asarray(inputs["bh"], F32) @ lhW + np.asarray(inputs["lh_b"], F32)
    W1 = np.asarray(inputs["mlp_W1"], F32)
    b1 = np.asarray(inputs["mlp_b1"], F32)
    W2 = np.asarray(inputs["mlp_W2"], F32)
    b2 = np.asarray(inputs["mlp_b2"], F32)

    wfused = np.ascontiguousarray(np.concatenate([Wzp, Whp], 1)).astype(BF16)
    gbias = np.tile(np.concatenate([bzp, bhp])[None, :], (128, 1)).astype(F32)
    w1cat = np.ascontiguousarray(np.concatenate([W1[:64], W1[64:]], 1)).astype(BF16)
    b1row = np.concatenate([np.zeros(64, F32), b1])[None, :].astype(BF16)
    w2sb = W2.astype(BF16)
    b2rep = np.tile(b2[None, :], (128, 64)).astype(F32)
    iota = np.tile(np.arange(128, dtype=F32)[None, :], (128, 1))
    ident = np.eye(128, dtype=F32).astype(BF16)
    ones1 = np.ones((1, 128), BF16)

    xpad = np.zeros((NPAD, FIN), F32)
    xpad[:N] = x
    xts = [np.ascontiguousarray(xpad[k * NLOC:(k + 1) * NLOC].T).astype(BF16)
           for k in range(NCORES)]

    counts = np.bincount(dst, minlength=NPAD)
    L = int(counts.max())
    order = np.argsort(dst, kind="stable")
    dsts = dst[order]
    ews = ew[order]
    srcs = src[order]
    cum = np.zeros(NPAD + 1, np.int64)
    cum[1:] = np.cumsum(counts)
    posn = np.arange(E) - cum[dsts]
    ewpad = np.zeros((NPAD, L), F32)
    ewpad[dsts, posn] = ews
    ewpads = [np.ascontiguousarray(
        ewpad[k * NLOC:(k + 1) * NLOC]
        .reshape(TPC, 128, L).transpose(1, 0, 2).reshape(128, TPC * L))
        for k in range(NCORES)]

    # ---- aggregation streams (sorted by dst, sharded by dst owner) ----
    k_of = dsts // NLOC
    t_of = (dsts % NLOC) // 128
    cnt = np.zeros((NCORES, TPC), np.int64)
    np.add.at(cnt, (k_of, t_of), 1)
    caps = np.maximum((cnt.max(0) + 127) // 128, 1)

    ngroups = (TPC + GT - 1) // GT
    groups = [list(range(g * GT, min((g + 1) * GT, TPC))) for g in range(ngroups)]
    colof = np.zeros(TPC, np.int64)
    gcol = []
    c = 0
    for grp in groups:
        c0 = c
        for t in grp:
            colof[t] = c
            c += int(caps[t])
        gcol.append((c0, c - c0))
    CTOT = c

    agg = []
    for k in range(NCORES):
        sel = slice(cum[k * NLOC], cum[(k + 1) * NLOC])
        d_k = dsts[sel]
        s_k = srcs[sel]
        e_k = ews[sel]
        t_k = (d_k - k * NLOC) // 128
        bstart = np.zeros(TPC, np.int64)
        bstart[1:] = np.cumsum(cnt[k])[:-1]
        pos = np.arange(len(t_k)) - bstart[t_k]
        col = colof[t_k] + pos // 128
        par = pos % 128
        gidx = np.zeros((128, SP * CTOT), np.int32)
        dstrel = np.zeros((128, CTOT), F32)
        ewagg = np.zeros((128, CTOT), F32)
        gidx[par, SP * col] = s_k.astype(np.int32)
        dstrel[par, col] = ((d_k - k * NLOC) % 128).astype(F32)
        ewagg[par, col] = e_k
        agg.append((gidx, dstrel, ewagg))

    # ---- MLP streams: sharded by edst owner, sorted by edst; B expanded ----
    morder = np.argsort(edst, kind="stable")
    medst = edst[morder]
    mesrc = esrc[morder]
    mcum = np.zeros(NPAD + 1, np.int64)
    mcum[1:] = np.cumsum(np.bincount(medst, minlength=NPAD))
    mk_of = medst // NLOC
    mt_of = (medst % NLOC) // 128
    cntm = np.zeros((NCORES, TPC), np.int64)
    np.add.at(cntm, (mk_of, mt_of), 1)
    capm = (cntm.max(0) + 127) // 128          # chunks per tile (0 allowed)
    mcolof = np.zeros(TPC, np.int64)
    c = 0
    for t in range(TPC):
        mcolof[t] = c
        c += int(capm[t])
    CTOTM = c
    mchunks = [(t, cc) for t in range(TPC) for cc in range(int(capm[t]))]

    mlp = []
    for k in range(NCORES):
        sel = slice(mcum[k * NLOC], mcum[(k + 1) * NLOC])
        d_k = medst[sel]
        s_k = mesrc[sel]
        t_k = (d_k - k * NLOC) // 128
        bstart = np.zeros(TPC, np.int64)
        bstart[1:] = np.cumsum(cntm[k])[:-1]
        pos = np.arange(len(t_k)) - bstart[t_k]
        col = mcolof[t_k] + pos // 128
        par = pos % 128
        ms = np.zeros((128, SP * CTOTM), np.int32)
        drm = np.zeros((128, CTOTM), np.float32)
        perm = np.full(CTOTM * 128, -1, np.int64)
        ms[par, SP * col] = (2 * s_k).astype(np.int32)
        drm[par, col] = ((d_k - k * NLOC) % 128).astype(np.float32)
        perm[col * 128 + par] = morder[sel]
        mlp.append((ms, drm.astype(BF16), perm))

    NPIECE = (CTOTM + 63) // 64                 # output groups of 64 chunks

    iotap = np.arange(128, dtype=F32).reshape(128, 1)
    consts = dict(wfused=wfused, gbias=gbias, w1cat=w1cat, b1row=b1row,
                  w2sb=w2sb, b2rep=b2rep, iota=iota, iotap=iotap, ident=ident,
                  ones1=ones1)
    in_maps = []
    for k in range(NCORES):
        gidx, dstrel, ewagg = agg[k]
        ms, drow, _ = mlp[k]
        in_maps.append(dict(
            xt=xts[k], ewpad=ewpads[k], gidx=gidx, dstrel=dstrel, ewagg=ewagg,
            masrc=ms, mdrow=drow, **consts))

    meta = dict(L=L, caps=caps, groups=groups, colof=colof, gcol=gcol,
                CTOT=CTOT, CTOTM=CTOTM, capm=capm, mcolof=mcolof,
                mchunks=mchunks, NPIECE=NPIECE,
                perms=[m[2] for m in mlp])
    return in_maps, meta


def _build(meta):
    L = meta["L"]
    caps = meta["caps"]
    groups = meta["groups"]
    colof = meta["colof"]
    gcol = meta["gcol"]
    CTOT = meta["CTOT"]

    nc = bacc.Bacc("TRN2", target_bir_lowering=False, debug=False,
                   num_devices=NCORES)

    xt_d = nc.dram_tensor("xt", [FIN, NLOC], dt.bfloat16, kind="ExternalInput")
    ewpad_d = nc.dram_tensor("ewpad", [128, TPC * L], dt.float32, kind="ExternalInput")
    gidx_d = nc.dram_tensor("gidx", [128, SP * CTOT], dt.int32, kind="ExternalInput")
    dstrel_d = nc.dram_tensor("dstrel", [128, CTOT], dt.float32, kind="ExternalInput")
    ewagg_d = nc.dram_tensor("ewagg", [128, CTOT], dt.float32, kind="ExternalInput")
    CTOTM = meta["CTOTM"]
    capm = meta["capm"]
    mcolof = meta["mcolof"]
    mchunks = meta["mchunks"]
    NPIECE = meta["NPIECE"]
    masrc_d = nc.dram_tensor("masrc", [128, SP * CTOTM], dt.int32, kind="ExternalInput")
    mdrow_d = nc.dram_tensor("mdrow", [128, CTOTM], dt.bfloat16, kind="ExternalInput")
    iotap_d = nc.dram_tensor("iotap", [128, 1], dt.float32, kind="ExternalInput")
    wfused_d = nc.dram_tensor("wfused", [FIN, 128], dt.bfloat16, kind="ExternalInput")
    gbias_d = nc.dram_tensor("gbias", [128, 128], dt.float32, kind="ExternalInput")
    w1cat_d = nc.dram_tensor("w1cat", [64, 128], dt.bfloat16, kind="ExternalInput")
    b1row_d = nc.dram_tensor("b1row", [1, 128], dt.bfloat16, kind="ExternalInput")
    w2sb_d = nc.dram_tensor("w2sb", [64, 2], dt.bfloat16, kind="ExternalInput")
    b2rep_d = nc.dram_tensor("b2rep", [128, 128], dt.float32, kind="ExternalInput")
    iota_d = nc.dram_tensor("iota", [128, 128], dt.float32, kind="ExternalInput")
    ident_d = nc.dram_tensor("ident", [128, 128], dt.bfloat16, kind="ExternalInput")
    ones1_d = nc.dram_tensor("ones1", [1, 128], dt.bfloat16, kind="ExternalInput")
    out_d = nc.dram_tensor("out", [NPIECE * 128, 128], dt.float32, kind="ExternalOutput")

    ysloc = nc.dram_tensor("ysloc", [NLOC, 128], dt.bfloat16)
    ysfull = nc.dram_tensor("ysfull", [NPAD, 128], dt.bfloat16, addr_space="Shared")
    abloc = nc.dram_tensor("abloc", [NLOC, 128], dt.bfloat16)
    abfull = nc.dram_tensor("abfull", [NPAD, 128], dt.bfloat16, addr_space="Shared")

    rg = [list(range(NCORES))]
    IOA = bass.IndirectOffsetOnAxis

    with tile.TileContext(nc) as tc:
        with (
            tc.tile_pool(name="const", bufs=1) as cp,
            tc.tile_pool(name="persist", bufs=1) as pp,
        ):
            wfused_sb = cp.tile([FIN, 128], dt.bfloat16)
            gbias_sb = cp.tile([128, 128], dt.float32)
            w1cat_sb = cp.tile([64, 128], dt.bfloat16)
            b1row_sb = cp.tile([1, 128], dt.bfloat16)
            w2_sb = cp.tile([64, 2], dt.bfloat16)
            iotap_sb = cp.tile([128, 1], dt.float32)
            b2rep_sb = cp.tile([128, 128], dt.float32)
            iota_sb = cp.tile([128, 128], dt.float32)
            ident_sb = cp.tile([128, 128], dt.bfloat16)
            ones1_sb = cp.tile([1, 128], dt.bfloat16)
            for sb, d in [
                (wfused_sb, wfused_d), (gbias_sb, gbias_d), (w1cat_sb, w1cat_d),
                (b1row_sb, b1row_d), (w2_sb, w2sb_d), (b2rep_sb, b2rep_d),
                (iota_sb, iota_d), (ident_sb, ident_d), (ones1_sb, ones1_d),
                (iotap_sb, iotap_d),
            ]:
                nc.sync.dma_start(out=sb[:], in_=d[:])

            dinv = pp.tile([128, TPC], dt.float32)
            dinv2 = pp.tile([128, TPC], dt.float32)
            ys2b = pp.tile([128, TPC * 128], dt.bfloat16)

            # ---------------- deg / dinv ----------------
            with tc.tile_pool(name="degp", bufs=1) as dp:
                ewpad_sb = dp.tile([128, TPC * L], dt.float32)
                nc.sync.dma_start(out=ewpad_sb[:], in_=ewpad_d[:])
                deg = dp.tile([128, TPC], dt.float32)
                nc.vector.tensor_reduce(
                    deg[:], ewpad_sb[:].rearrange("p (t l) -> p t l", t=TPC),
                    axis=mybir.AxisListType.X, op=mybir.AluOpType.add)
                sq = dp.tile([128, TPC], dt.float32)
                nc.scalar.activation(sq[:], deg[:],
                                     mybir.ActivationFunctionType.Sqrt, bias=1.0)
                nc.vector.reciprocal(dinv[:], sq[:])
                nc.vector.tensor_mul(dinv2[:], dinv[:], dinv[:])

            # ---------------- node phase ----------------
            with (
                tc.tile_pool(name="xtp", bufs=3) as xtp,
                tc.tile_pool(name="ysp", bufs=3) as ysp,
                tc.tile_pool(name="pY", bufs=2, space="PSUM") as pYp,
            ):
                for t in range(TPC):
                    xt_t = xtp.tile([FIN, 128], dt.bfloat16)
                    nc.sync.dma_start(out=xt_t[:], in_=xt_d[:, t * 128:(t + 1) * 128])
                    pY = pYp.tile([128, 128], dt.float32)
                    nc.tensor.matmul(pY[:], xt_t[:], wfused_sb[:], start=True, stop=True)
                    ys_t = ysp.tile([128, 128], dt.bfloat16)
                    nc.vector.tensor_scalar_mul(ys_t[:], pY[:], dinv[:, t:t + 1])
                    nc.vector.scalar_tensor_tensor(
                        ys2b[:, t * 128:(t + 1) * 128], pY[:], dinv2[:, t:t + 1],
                        gbias_sb[:], op0=mybir.AluOpType.mult, op1=mybir.AluOpType.add)
                    nc.sync.dma_start(out=ysloc[t * 128:(t + 1) * 128, :], in_=ys_t[:])

            nc.gpsimd.collective_compute(
                "AllGather", mybir.AluOpType.bypass, replica_groups=rg,
                ins=[ysloc[:]], outs=[ysfull[:]])

            # ---------------- aggregation + gates + A|B ----------------
            with (
                tc.tile_pool(name="aggstream", bufs=1) as asp,
                tc.tile_pool(name="goff", bufs=3) as gop,
                tc.tile_pool(name="gat", bufs=10) as gatp,
                tc.tile_pool(name="oh", bufs=8) as ohp,
                tc.tile_pool(name="gate", bufs=3) as gp_,
                tc.tile_pool(name="abp", bufs=3) as abp,
                tc.tile_pool(name="pAgg", bufs=GT, space="PSUM") as pAgg,
                tc.tile_pool(name="pT", bufs=1, space="PSUM") as pT,
                tc.tile_pool(name="pAB", bufs=1, space="PSUM") as pAB,
            ):
                dstrel_sb = asp.tile([128, CTOT], dt.float32)
                ewagg_sb = asp.tile([128, CTOT], dt.float32)
                nc.sync.dma_start(out=dstrel_sb[:], in_=dstrel_d[:])
                nc.sync.dma_start(out=ewagg_sb[:], in_=ewagg_d[:])

                for g, grp in enumerate(groups):
                    c0, Kg = gcol[g]
                    goff = gop.tile([128, SP * Kg], dt.int32, tag="goff")
                    nc.sync.dma_start(
                        out=goff[:], in_=gidx_d[:, SP * c0:SP * (c0 + Kg)])
                    for t in grp:
                        ps = pAgg.tile([128, 128], dt.float32, tag="aggpsum")
                        nch = int(caps[t])
                        for cc in range(nch):
                            col = int(colof[t]) + cc
                            lc = col - c0
                            gt_ = gatp.tile([128, 128], dt.bfloat16, tag="gat")
                            nc.gpsimd.indirect_dma_start(
                                out=gt_[:], out_offset=None, in_=ysfull[:],
                                in_offset=IOA(ap=goff[:, SP * lc:SP * lc + 1],
                                              axis=0))
                            oh = ohp.tile([128, 128], dt.bfloat16)
                            nc.vector.tensor_scalar(
                                oh[:], iota_sb[:], dstrel_sb[:, col:col + 1],
                                ewagg_sb[:, col:col + 1],
                                op0=mybir.AluOpType.is_equal,
                                op1=mybir.AluOpType.mult)
                            nc.tensor.matmul(ps[:], oh[:], gt_[:],
                                             start=cc == 0, stop=cc == nch - 1)
                        yagg = gp_.tile([128, 128], dt.float32, tag="yagg")
                        nc.vector.scalar_tensor_tensor(
                            yagg[:], ps[:], dinv[:, t:t + 1],
                            ys2b[:, t * 128:(t + 1) * 128],
                            op0=mybir.AluOpType.mult, op1=mybir.AluOpType.add)
                        zt = gp_.tile([128, 64], dt.float32, tag="zt")
                        ht = gp_.tile([128, 64], dt.float32, tag="ht")
                        nc.scalar.activation(zt[:], yagg[:, 0:64],
                                             mybir.ActivationFunctionType.Sigmoid)
                        nc.scalar.activation(ht[:], yagg[:, 64:128],
                                             mybir.ActivationFunctionType.Tanh)
                        zh = gp_.tile([128, 64], dt.float32, tag="zh")
                        nc.vector.tensor_mul(zh[:], zt[:], ht[:])
                        hbf = gp_.tile([128, 64], dt.bfloat16, tag="hbf")
                        nc.vector.tensor_sub(hbf[:], ht[:], zh[:])
                        psT = pT.tile([64, 128], dt.bfloat16)
                        nc.tensor.transpose(psT[:], hbf[:], ident_sb[:])
                        hT = gp_.tile([64, 128], dt.bfloat16, tag="hT")
                        nc.vector.tensor_copy(hT[:], psT[:])
                        psAB = pAB.tile([128, 128], dt.float32)
                        nc.tensor.matmul(psAB[:], ones1_sb[:], b1row_sb[:],
                                         start=True, stop=False)
                        nc.tensor.matmul(psAB[:], hT[:], w1cat_sb[:],
                                         start=False, stop=True)
                        ab = abp.tile([128, 128], dt.bfloat16)
                        nc.scalar.copy(ab[:], psAB[:])
                        nc.sync.dma_start(out=abloc[t * 128:(t + 1) * 128, :],
                                          in_=ab[:])

            nc.gpsimd.collective_compute(
                "AllGather", mybir.AluOpType.bypass, replica_groups=rg,
                ins=[abloc[:]], outs=[abfull[:]])

            # ---------------- MLP phase (A gathered, B expanded) ----------------
            abhalf = abfull[:].rearrange("n (a b) -> (n a) b", a=2)
            with (
                tc.tile_pool(name="mstream", bufs=1) as msp,
                tc.tile_pool(name="moff", bufs=3) as mop,
                tc.tile_pool(name="bwinp", bufs=3) as bwp,
                tc.tile_pool(name="sgp", bufs=10) as sgp,
                tc.tile_pool(name="ohn", bufs=6) as ohnp,
                tc.tile_pool(name="hT2", bufs=8) as hT2p,
                tc.tile_pool(name="op", bufs=3) as op_,
                tc.tile_pool(name="pO", bufs=2, space="PSUM") as pOp,
                tc.tile_pool(name="pBC", bufs=3, space="PSUM") as pBCp,
                tc.tile_pool(name="pE", bufs=3, space="PSUM") as pEp,
            ):
                mdrow_sb = msp.tile([128, CTOTM], dt.bfloat16)
                nc.sync.dma_start(out=mdrow_sb[:], in_=mdrow_d[:])
                MG = 64
                pO = None
                bwin = None
                cur_t = -1
                moff = None
                moff_base = 0
                for cg, (t, cc) in enumerate(mchunks):
                    b = cg % MG
                    if b == 0:
                        pO = pOp.tile([128, 128], dt.float32)
                        moff = mop.tile([128, SP * MG], dt.int32, tag="moff")
                        moff_base = cg
                        nw = min(MG, CTOTM - cg)
                        nc.sync.dma_start(
                            out=moff[:, :SP * nw],
                            in_=masrc_d[:, SP * cg:SP * (cg + nw)])
                    if t != cur_t:
                        bwin = bwp.tile([128, 64], dt.bfloat16, tag="bwin")
                        nc.sync.dma_start(
                            out=bwin[:],
                            in_=abloc[t * 128:(t + 1) * 128, 64:128])
                        cur_t = t
                    lc = cg - moff_base
                    ag = sgp.tile([128, 64], dt.bfloat16, tag="ag")
                    nc.gpsimd.indirect_dma_start(
                        out=ag[:], out_offset=None, in_=abhalf,
                        in_offset=IOA(ap=moff[:, SP * lc:SP * lc + 1], axis=0))
                    psBC = pBCp.tile([128, 128], dt.bfloat16)
                    nc.tensor.transpose(
                        psBC[:],
                        mdrow_sb[:, cg:cg + 1].to_broadcast([128, 128]),
                        ident_sb[:])
                    ohn = ohnp.tile([128, 128], dt.bfloat16, tag="ohn")
                    nc.vector.tensor_scalar(
                        ohn[:], psBC[:], iotap_sb[:, 0:1], None,
                        op0=mybir.AluOpType.is_equal)
                    psE = pEp.tile([64, 128], dt.float32)
                    nc.tensor.matmul(psE[:], bwin[:], ohn[:],
                                     start=True, stop=False)
                    nc.tensor.matmul(psE[:], ag[:], ident_sb[:],
                                     start=False, stop=True)
                    hTm = hT2p.tile([64, 128], dt.bfloat16)
                    nc.scalar.activation(hTm[:], psE[:],
                                         mybir.ActivationFunctionType.Relu)
                    nc.tensor.matmul(pO[:, 2 * b:2 * b + 2], hTm[:], w2_sb[:],
                                     start=True, stop=True)
                    if b == MG - 1 or cg == CTOTM - 1:
                        nb = b + 1
                        osb = op_.tile([128, 128], dt.float32)
                        nc.vector.tensor_add(osb[:, :2 * nb], pO[:, :2 * nb],
                                             b2rep_sb[:, :2 * nb])
                        q = cg // MG
                        nc.sync.dma_start(
                            out=out_d[q * 128:(q + 1) * 128, :], in_=osb[:])

    nc.compile()
    return nc


def _unshard(results, meta):
    CTOTM = meta["CTOTM"]
    out = np.zeros((E, 2), F32)
    for k in range(NCORES):
        outd = np.asarray(results[k]["out"])
        nslots = CTOTM * 128
        stream = np.empty((nslots, 2), F32)
        for q in range((CTOTM + 63) // 64):
            kk = min(64, CTOTM - q * 64)
            blk = outd[q * 128:(q + 1) * 128, :2 * kk]
            stream[q * 64 * 128:(q * 64 + kk) * 128] = (
                blk.reshape(128, kk, 2).transpose(1, 0, 2).reshape(kk * 128, 2))
        perm = meta["perms"][k]
        valid = perm >= 0
        out[perm[valid]] = stream[valid]
    return out


def kernel(**inputs):
    in_maps, meta = _prep(inputs)
    nc = _build(meta)
    res = run_bass_kernel_spmd(nc, in_maps, list(range(NCORES)))
    return _unshard(res.results, meta)
